# revision 10
# baseline (speedup 1.0000x reference)
"""Biased axial (tied) attention kernel for 8 Trainium2 NeuronCores.

Sharding: scores contract over the first L axis (n).  Each core takes 48
n-rows, computes partial scores A[i,h,j] for ALL (i,j); a ReduceScatter
sums the partials and hands each core only its own 48 i-rows.  Each core
softmaxes just those rows (bias projection bp[i in shard, j] is computed
locally from the matching bias slice - no gather), and an AllGather of the
softmaxed bf16 attention rebuilds the full [384,4,384] attn on every core.
The output einsum computes out columns k in the core's n-shard, so final
output rows are shard-contiguous.

The gate tensor sigmoid(LN(p)[i, k in shard] @ Wg) needs LN'd pair rows
owned by other cores; instead of re-loading + re-normalizing the
transposed pair slice, an AllToAll exchanges column slices of the already
LayerNormed feature-major pair shard (each core sends [d, 48 n, 48 k]
blocks), eliminating the third LayerNorm entirely.

Compute layout: LayerNorm runs position-major with grouped bn_stats (4
positions per instr), the apply pass is spread over vector/scalar/gpsimd,
and normalized bf16 tiles flip to feature-major via batched DMA-xbar
transposes.  V is produced directly j-partition-major by using the LN'd
pair block as the matmul stationary operand (no V transpose).  All
matmuls run bf16 with fp32 PSUM accumulation; scores use K=32 row-tiled
PE packing (4 heads concurrent), the output einsum M=32 col-tiled packing.
"""

import functools
import math
from contextlib import ExitStack

import numpy as np
import ml_dtypes

import concourse.bacc as bacc
import concourse.mybir as mybir
from concourse.bass_utils import run_bass_kernel_spmd
from concourse.tile import TileContext

N_CORES = 8
L = 384
D = 128
H = 4
DH = 32
NL = L // N_CORES          # 48 rows per core
NCHUNK = L // 128          # 3
NPOS = L * NL              # 18432 positions per LN'd tensor slice
EPS = 1e-5

F32 = mybir.dt.float32
BF16 = mybir.dt.bfloat16
AX = mybir.AxisListType
ALU = mybir.AluOpType
ACTF = mybir.ActivationFunctionType

RG = [list(range(N_CORES))]
GROUPED_STATS = False


def _emit_ln(nc, pools, src, lnfm_out, s6tag):
    """LayerNorm `src` (DRAM f32 [L, NL, D]) position-major; write bf16
    feature-major chunks into lnfm_out(cc) ([128, 48, 128] AP per chunk)."""
    s6p, mvp, xccp = pools
    s6 = s6p.tile([128, NCHUNK, 48, 6], F32, tag=s6tag)
    mean = mvp.tile([128, NCHUNK * 48], F32, tag="mean")
    negm = mvp.tile([128, NCHUNK * 48], F32, tag="negm")
    tA = mvp.tile([128, NCHUNK * 48], F32, tag="tA")
    tB = mvp.tile([128, NCHUNK * 48], F32, tag="tB")
    rs = mvp.tile([128, NCHUNK * 48], F32, tag="rs")
    for cc in range(NCHUNK):
        xcc = xccp.tile([128, 48, D], BF16, tag="xcc")
        nc.gpsimd.dma_start(out=xcc[:], in_=src[cc * 128:(cc + 1) * 128, :, :])
        if GROUPED_STATS:
            for g in range(12):
                nc.vector.bn_stats(out=s6[:, cc, 4 * g:4 * (g + 1), :],
                                   in_=xcc[:, 4 * g:4 * (g + 1), :])
        else:
            for n in range(48):
                nc.vector.bn_stats(out=s6[:, cc, n, :], in_=xcc[:, n, :])
        # batched stats post-processing for this cc.
        # bn_stats 6-tuple = (cnt, mean, cnt*var) of even / odd elements.
        sl = slice(cc * 48, (cc + 1) * 48)
        me = s6[:, cc, :, 1]
        mo = s6[:, cc, :, 4]
        cve = s6[:, cc, :, 2]
        cvo = s6[:, cc, :, 5]
        nc.vector.tensor_add(out=tA[:, sl], in0=me, in1=mo)
        nc.vector.tensor_scalar_mul(mean[:, sl], tA[:, sl], 0.5)
        nc.vector.tensor_scalar_mul(negm[:, sl], mean[:, sl], -1.0)
        # var = (cve+cvo)/128 + (me^2+mo^2)/2 - mean^2
        nc.vector.tensor_add(out=tA[:, sl], in0=cve, in1=cvo)
        nc.vector.tensor_scalar_mul(tA[:, sl], tA[:, sl], 1.0 / 128.0)
        nc.vector.tensor_mul(out=tB[:, sl], in0=me, in1=me)
        nc.vector.scalar_tensor_tensor(
            out=tA[:, sl], in0=tB[:, sl], scalar=0.5, in1=tA[:, sl],
            op0=ALU.mult, op1=ALU.add)
        nc.vector.tensor_mul(out=tB[:, sl], in0=mo, in1=mo)
        nc.vector.scalar_tensor_tensor(
            out=tA[:, sl], in0=tB[:, sl], scalar=0.5, in1=tA[:, sl],
            op0=ALU.mult, op1=ALU.add)
        nc.vector.tensor_mul(out=tB[:, sl], in0=mean[:, sl], in1=mean[:, sl])
        nc.vector.tensor_sub(out=tA[:, sl], in0=tA[:, sl], in1=tB[:, sl])
        # rs = 1/sqrt(var+eps)
        nc.vector.tensor_scalar_add(tA[:, sl], tA[:, sl], EPS)
        nc.scalar.sqrt(out=tB[:, sl], in_=tA[:, sl])
        nc.vector.reciprocal(out=rs[:, sl], in_=tB[:, sl])
        nc.vector.tensor_mul(out=tB[:, sl], in0=negm[:, sl], in1=rs[:, sl])
        # apply (in place), spread across vector/scalar/gpsimd
        for n in range(48):
            col = cc * 48 + n
            m = n % 4
            if m == 2:
                nc.scalar.activation(
                    xcc[:, n, :], xcc[:, n, :], ACTF.Identity,
                    bias=tB[:, col:col + 1], scale=rs[:, col:col + 1])
            elif m == 3:
                nc.gpsimd.tensor_scalar(
                    out=xcc[:, n, :], in0=xcc[:, n, :],
                    scalar1=negm[:, col:col + 1], scalar2=rs[:, col:col + 1],
                    op0=ALU.add, op1=ALU.mult)
            else:
                nc.vector.tensor_scalar(
                    out=xcc[:, n, :], in0=xcc[:, n, :],
                    scalar1=negm[:, col:col + 1], scalar2=rs[:, col:col + 1],
                    op0=ALU.add, op1=ALU.mult)
        nc.sync.dma_start(
            out=lnfm_out(cc),
            in_=xcc.rearrange("p n j -> p (n j)"), transpose=True)


@functools.lru_cache(maxsize=4)
def build_program(has_bo: bool, has_cv: bool):
    nc = bacc.Bacc(num_devices=N_CORES)

    xr = nc.declare_dram_parameter("xr", [L, NL, D], F32, isOutput=False)
    xb = nc.declare_dram_parameter("xb", [L, NL, D], F32, isOutput=False)
    wq = nc.declare_dram_parameter("wq", [D, D], BF16, isOutput=False)
    wk = nc.declare_dram_parameter("wk", [D, D], BF16, isOutput=False)
    wv = nc.declare_dram_parameter("wv", [D, D], BF16, isOutput=False)
    wg = nc.declare_dram_parameter("wg", [D, D], BF16, isOutput=False)
    wo = nc.declare_dram_parameter("wo", [D, D], BF16, isOutput=False)
    wb = nc.declare_dram_parameter("wb", [D, H], BF16, isOutput=False)
    cq = nc.declare_dram_parameter("cq", [D, 1], F32, isOutput=False)
    ck = nc.declare_dram_parameter("ck", [D, 1], F32, isOutput=False)
    cvb = nc.declare_dram_parameter("cvb", [D, D], F32, isOutput=False)
    cg = nc.declare_dram_parameter("cg", [D, 1], F32, isOutput=False)
    cb = nc.declare_dram_parameter("cb", [H, 1], F32, isOutput=False)
    bo_b = nc.declare_dram_parameter("bo_b", [D, D], F32, isOutput=False)
    out = nc.declare_dram_parameter("out", [NL, L, D], F32, isOutput=True)

    a_part = nc.dram_tensor("a_part", [L, H, L], F32)
    a_rs = nc.dram_tensor("a_rs", [NL, H, L], F32)
    at_loc = nc.dram_tensor("at_loc", [NL, H * L], BF16)
    at_gth = nc.dram_tensor("at_gth", [N_CORES, NL, H * L], BF16,
                            addr_space="Shared")
    g_send = nc.dram_tensor("g_send", [N_CORES, D, NL, NL], BF16)
    g_recv = nc.dram_tensor("g_recv", [N_CORES, D, NL, NL], BF16)
    bp_dram = nc.dram_tensor("bp_dram", [H, NCHUNK, 48, 128], BF16)
    junk_d = nc.dram_tensor("junk_d", [128, 64], F32)

    with TileContext(nc) as tc, ExitStack() as es:
        cpool = es.enter_context(tc.tile_pool(name="consts", bufs=1))
        wq_sb = cpool.tile([D, D], BF16, tag="wq")
        wk_sb = cpool.tile([D, D], BF16, tag="wk")
        wv_sb = cpool.tile([D, D], BF16, tag="wv")
        wg_sb = cpool.tile([D, D], BF16, tag="wg")
        wo_sb = cpool.tile([D, D], BF16, tag="wo")
        wb_sb = cpool.tile([D, H], BF16, tag="wb")
        cq_sb = cpool.tile([D, 1], F32, tag="cq")
        ck_sb = cpool.tile([D, 1], F32, tag="ck")
        cg_sb = cpool.tile([D, 1], F32, tag="cg")
        cb_sb = cpool.tile([H, 1], F32, tag="cb")
        for t, s in [(wq_sb, wq), (wk_sb, wk), (wv_sb, wv), (wg_sb, wg),
                     (wo_sb, wo), (wb_sb, wb), (cq_sb, cq), (ck_sb, ck),
                     (cg_sb, cg), (cb_sb, cb)]:
            nc.sync.dma_start(out=t[:], in_=s[:])
        if has_bo:
            bo_sb = cpool.tile([D, D], F32, tag="bo")
            nc.sync.dma_start(out=bo_sb[:], in_=bo_b[:])
        if has_cv:
            cvb_sb = cpool.tile([D, 4, D], F32, tag="cvb")
            for g in range(4):
                nc.sync.dma_start(out=cvb_sb[:, g, :], in_=cvb[:])

        # ---- phase 0: PE warmup (HAM un-throttle before real matmuls)
        with tc.tile_pool(name="wup", bufs=1, space="PSUM") as wup, \
             tc.tile_pool(name="wus", bufs=1) as wus:
            wps = wup.tile([128, 64], F32, tag="wps")
            for i in range(40):
                nc.tensor.matmul(wps[:], lhsT=wq_sb[:], rhs=wk_sb[:, 0:64],
                                 start=True, stop=True)
            wsb = wus.tile([128, 64], F32, tag="wsb")
            nc.vector.tensor_copy(wsb[:], wps[:])
            nc.gpsimd.dma_start(out=junk_d[:], in_=wsb[:])

        bigp = es.enter_context(tc.tile_pool(name="big", bufs=1))
        es_ln = ExitStack()
        s6p = es_ln.enter_context(tc.tile_pool(name="s6", bufs=1))
        mvp = es_ln.enter_context(tc.tile_pool(name="mv", bufs=1))
        xccp = es_ln.enter_context(tc.tile_pool(name="xcc", bufs=2))
        lnrp = es_ln.enter_context(tc.tile_pool(name="lnr", bufs=1))
        ln_pools = (s6p, mvp, xccp)

        # ---- phase 1: LN of pair n-shard -> lnr[cc] (feature-major bf16)
        lnr = [lnrp.tile([128, 48, 128], BF16, tag=f"lnr{cc}", name=f"lnr{cc}")
               for cc in range(NCHUNK)]
        _emit_ln(nc, ln_pools, xr, lambda cc: lnr[cc][:], "s6r")

        k_sb = bigp.tile([128, NCHUNK, 48, 128], BF16, tag="bigk")
        v_pm = bigp.tile([128, NL, NCHUNK, 128], BF16, tag="bigv")

        with tc.tile_pool(name="qp", bufs=1) as qpool:
            # ---- phase 2: q/k projections (feature-major)
            q_sb = [qpool.tile([128, 48, 128], BF16, tag=f"q{cc}", name=f"q{cc}")
                    for cc in range(NCHUNK)]
            with tc.tile_pool(name="qkps", bufs=4, space="PSUM") as qkps:
                for cc in range(NCHUNK):
                    lncc = lnr[cc][:].rearrange("p n j -> p (n j)")
                    qcc = q_sb[cc][:].rearrange("p n j -> p (n j)")
                    kcc = k_sb[:, cc, :, :].rearrange("p n j -> p (n j)")
                    for ch in range(12):
                        sl = slice(ch * 512, (ch + 1) * 512)
                        ps = qkps.tile([128, 512], F32, tag="qps")
                        nc.tensor.matmul(ps[:], lhsT=wq_sb[:], rhs=lncc[:, sl],
                                         start=True, stop=True)
                        nc.scalar.activation(qcc[:, sl], ps[:], ACTF.Identity,
                                             bias=cq_sb[:, 0:1])
                    for ch in range(12):
                        sl = slice(ch * 512, (ch + 1) * 512)
                        ps = qkps.tile([128, 512], F32, tag="kps")
                        nc.tensor.matmul(ps[:], lhsT=wk_sb[:], rhs=lncc[:, sl],
                                         start=True, stop=True)
                        nc.vector.tensor_scalar_add(kcc[:, sl], ps[:],
                                                    ck_sb[:, 0:1])

            # ---- phase 2b: AllToAll staging for the gate source (gpsimd/scalar)
            with tc.tile_pool(name="a2as", bufs=1) as a2ap:
                for dst in range(N_CORES):
                    j0g = dst * NL
                    st = a2ap.tile([128, 48, 48], BF16, tag="a2a")
                    cc0, o0 = divmod(j0g, 128)
                    w0 = min(128 - o0, 48)
                    eng = nc.gpsimd if dst % 2 == 0 else nc.scalar
                    if eng is nc.scalar:
                        eng.copy(st[:, :, 0:w0], lnr[cc0][:, :, o0:o0 + w0])
                    else:
                        eng.tensor_copy(st[:, :, 0:w0], lnr[cc0][:, :, o0:o0 + w0])
                    if w0 < 48:
                        nc.gpsimd.tensor_copy(st[:, :, w0:48],
                                              lnr[cc0 + 1][:, :, 0:48 - w0])
                    nc.gpsimd.dma_start(out=g_send[dst], in_=st[:])
            nc.gpsimd.collective_compute(
                "AllToAll", ALU.bypass, replica_groups=RG,
                ins=[g_send[:]], outs=[g_recv[:]])

            # ---- phase 3: scores A[i,h,j], K=32 row-tiled, 4 heads packed
            with tc.tile_pool(name="apsum", bufs=1, space="PSUM") as apsum, \
                 tc.tile_pool(name="asb", bufs=1) as asbp:
                for ic in range(NCHUNK):
                    aps = [apsum.tile([128, L], F32, tag=f"A{h}", name=f"A{h}")
                           for h in range(H)]
                    for n in range(NL):
                        for h in range(H):
                            nc.tensor.matmul(
                                aps[h][:],
                                lhsT=q_sb[ic][32 * h:32 * (h + 1), n, :],
                                rhs=k_sb[32 * h:32 * (h + 1), :, n, :],
                                start=(n == 0), stop=(n == NL - 1),
                                tile_position=(32 * h, 0))
                    a_sb = asbp.tile([128, H, L], F32, tag="asb")
                    for h in range(H):
                        if h % 2 == 0:
                            nc.vector.tensor_copy(a_sb[:, h, :], aps[h][:])
                        else:
                            nc.scalar.copy(a_sb[:, h, :], aps[h][:])
                    nc.scalar.dma_start(
                        out=a_part[ic * 128:(ic + 1) * 128], in_=a_sb[:])

            # ---- phase 5: v, j-partition-major via x-as-stationary matmuls
            with tc.tile_pool(name="vps", bufs=3, space="PSUM") as vpsp:
                vflat = v_pm[:].rearrange("p k b j -> p (k b) j")
                flat = 0
                while flat < NL * NCHUNK:
                    ps = vpsp.tile([128, 4, 128], F32, tag="vps")
                    for g in range(4):
                        k_, jb = divmod(flat + g, NCHUNK)
                        nc.tensor.matmul(ps[:, g, :], lhsT=lnr[jb][:, k_, :],
                                         rhs=wv_sb[:], start=True, stop=True)
                    dst = vflat[:, flat:flat + 4, :]
                    if has_cv:
                        nc.vector.tensor_add(out=dst, in0=ps[:], in1=cvb_sb[:])
                    elif flat % 8 == 0:
                        nc.vector.tensor_copy(dst, ps[:])
                    else:
                        nc.scalar.copy(dst, ps[:])
                    flat += 4

        # ---- phase 6: LN of bias shard -> bp (sharded by i; no gather)
        lnbp = es_ln.enter_context(tc.tile_pool(name="lnb", bufs=2))
        lnb_tiles = {}

        def lnb_out(cc):
            t = lnbp.tile([128, 48, 128], BF16, tag="lnb")
            lnb_tiles[cc] = t
            return t[:]

        _emit_ln(nc, ln_pools, xb, lnb_out, "s6b")
        with tc.tile_pool(name="bpps", bufs=4, space="PSUM") as bpps, \
             tc.tile_pool(name="bpsb", bufs=1) as bpsbp:
            for cc in range(NCHUNK):
                lncc = lnb_tiles[cc][:].rearrange("p n j -> p (n j)")
                bp_t = bpsbp.tile([H, 6144], BF16, tag="bpt")
                for ch in range(12):
                    sl = slice(ch * 512, (ch + 1) * 512)
                    ps = bpps.tile([H, 512], F32, tag="bpps")
                    nc.tensor.matmul(ps[:], lhsT=wb_sb[:], rhs=lncc[:, sl],
                                     start=True, stop=True)
                    if ch % 2 == 0:
                        nc.vector.tensor_scalar_add(bp_t[:, sl], ps[:],
                                                    cb_sb[:, 0:1])
                    else:
                        nc.scalar.activation(bp_t[:, sl], ps[:], ACTF.Identity,
                                             bias=cb_sb[:, 0:1])
                nc.sync.dma_start(out=bp_dram[:, cc],
                                  in_=bp_t.rearrange("h (n j) -> h n j", j=128))

        es_ln.close()

        # ---- phase 7: ReduceScatter of scores; softmax of own 48 i-rows
        nc.gpsimd.collective_compute(
            "ReduceScatter", ALU.add, replica_groups=RG,
            ins=[a_part[:]], outs=[a_rs[:]])

        with tc.tile_pool(name="smp", bufs=1) as smp, \
             tc.tile_pool(name="sms", bufs=1) as sms:
            a_ch = smp.tile([NL, H, L], F32, tag="ach")
            nc.sync.dma_start(out=a_ch[:], in_=a_rs[:])
            bp_raw = smp.tile([NL, H, NCHUNK, 128], BF16, tag="bpraw")
            nc.sync.dma_start(
                out=bp_raw[:],
                in_=bp_dram.rearrange("h c n j -> n h c j"))
            bp_f = smp.tile([NL, H, L], F32, tag="bpf")
            nc.vector.tensor_copy(
                bp_f[:], bp_raw[:].rearrange("p h c j -> p h (c j)"))
            nc.vector.tensor_add(out=a_ch[:], in0=a_ch[:], in1=bp_f[:])
            nm = sms.tile([NL, H], F32, tag="nm")
            nc.vector.tensor_reduce(out=nm[:], in_=a_ch[:], axis=AX.X,
                                    op=ALU.max)
            nmn = sms.tile([NL, H], F32, tag="nmn")
            nc.vector.tensor_scalar_mul(nmn[:], nm[:], -1.0)
            e_ch = smp.tile([NL, H, L], F32, tag="ech")
            ssum = sms.tile([NL, H], F32, tag="ssum")
            for h in range(H):
                nc.scalar.activation(e_ch[:, h, :], a_ch[:, h, :], ACTF.Exp,
                                     bias=nmn[:, h:h + 1],
                                     accum_out=ssum[:, h:h + 1])
            rsum = sms.tile([NL, H], F32, tag="rsum")
            nc.vector.reciprocal(out=rsum[:], in_=ssum[:])
            at_ch = smp.tile([NL, H, L], BF16, tag="atch")
            for h in range(H):
                nc.vector.tensor_scalar_mul(at_ch[:, h, :], e_ch[:, h, :],
                                            rsum[:, h:h + 1])
            nc.sync.dma_start(out=at_loc[:],
                              in_=at_ch.rearrange("p h j -> p (h j)"))

        nc.gpsimd.collective_compute(
            "AllGather", ALU.bypass, replica_groups=RG,
            ins=[at_loc[:]], outs=[at_gth[:]])

        # ---- phase 8: rebuild attn, transpose to j-major; gate projection
        attnT = bigp.tile([128, H, NCHUNK, NCHUNK, 128], BF16, tag="bigattnT")
        with tc.tile_pool(name="atl", bufs=2) as atlp:
            for ic in range(NCHUNK):
                at_in = atlp.tile([128, H * L], BF16, tag="atin")
                nc.sync.dma_start(
                    out=at_in[:],
                    in_=at_gth.rearrange("c n f -> (c n) f")
                        [ic * 128:(ic + 1) * 128, :])
                nc.sync.dma_start(
                    out=attnT[:, :, :, ic, :].rearrange("p h b j -> p (h b) j"),
                    in_=at_in[:], transpose=True)

        with tc.tile_pool(name="gatep", bufs=1) as gatepp, \
             tc.tile_pool(name="gfm", bufs=1) as gfmp:
            gate_sb = gatepp.tile([128, L, NL], BF16, tag="gate")
            with tc.tile_pool(name="gps", bufs=4, space="PSUM") as gpsp:
                g_fm = gfmp.tile([128, L, NL], BF16, tag="gfm")
                nc.sync.dma_start(
                    out=g_fm[:].rearrange("p (c n) k -> p c n k", c=N_CORES),
                    in_=g_recv.rearrange("c d n k -> d c n k"))
                gflat = g_fm[:].rearrange("p i k -> p (i k)")
                oflat = gate_sb[:].rearrange("p i k -> p (i k)")
                for ch in range(NPOS // 512):
                    sl = slice(ch * 512, (ch + 1) * 512)
                    ps = gpsp.tile([128, 512], F32, tag="gps")
                    nc.tensor.matmul(ps[:], lhsT=wg_sb[:], rhs=gflat[:, sl],
                                     start=True, stop=True)
                    nc.scalar.activation(oflat[:, sl], ps[:], ACTF.Sigmoid,
                                         bias=cg_sb[:, 0:1])

            # ---- phase 9: output einsum (col-tiled) + gate + out proj
            with tc.tile_pool(name="opsum", bufs=2, space="PSUM") as opsum, \
                 tc.tile_pool(name="fpsum", bufs=3, space="PSUM") as fpsum, \
                 tc.tile_pool(name="ogp", bufs=3) as ogp, \
                 tc.tile_pool(name="fsbp", bufs=2) as fsbp:
                for k_ in range(NL):
                    ops_t = opsum.tile([128, L], F32, tag="ops")
                    for jb in range(NCHUNK):
                        for h in range(H):
                            nc.tensor.matmul(
                                ops_t[32 * h:32 * (h + 1), :],
                                lhsT=v_pm[:, k_, jb, 32 * h:32 * (h + 1)],
                                rhs=attnT[:, h, jb, :, :],
                                start=(jb == 0), stop=(jb == NCHUNK - 1),
                                tile_position=(0, 32 * h))
                    og = ogp.tile([128, L], BF16, tag="og")
                    nc.vector.scalar_tensor_tensor(
                        out=og[:], in0=ops_t[:], scalar=1.0,
                        in1=gate_sb[:, :, k_],
                        op0=ALU.mult, op1=ALU.mult)
                    fps = fpsum.tile([128, NCHUNK, D], F32, tag="fps")
                    for pc in range(NCHUNK):
                        nc.tensor.matmul(fps[:, pc, :],
                                         lhsT=og[:, pc * 128:(pc + 1) * 128],
                                         rhs=wo_sb[:], start=True, stop=True)
                    if k_ % 4 == 0:
                        fsb4 = fsbp.tile([128, 4, NCHUNK, D], F32, tag="fsb")
                    kk = k_ % 4
                    if has_bo:
                        for pc in range(NCHUNK):
                            nc.vector.tensor_add(out=fsb4[:, kk, pc, :],
                                                 in0=fps[:, pc, :],
                                                 in1=bo_sb[:])
                    else:
                        nc.scalar.copy(fsb4[:, kk, :, :], fps[:])
                    if kk == 3:
                        k0 = k_ - 3
                        nc.sync.dma_start(
                            out=out[k0:k0 + 4]
                                .rearrange("k (pc p) d -> p (k pc) d", p=128),
                            in_=fsb4.rearrange("p k pc d -> p (k pc) d"))

    nc.compile()
    return nc


def _prep_inputs(pair, bias, ln_pair_w, ln_pair_b, ln_bias_w, ln_bias_b,
                 Wq, Wk, Wv, Wb, Wg, bg, Wo, bo):
    bf = ml_dtypes.bfloat16
    scaling = 1.0 / math.sqrt(DH)
    kscale = 1.0 / math.sqrt(L)
    wq_e = (ln_pair_w[:, None] * Wq * scaling).astype(bf)
    wk_e = (ln_pair_w[:, None] * Wk * kscale).astype(bf)
    wv_e = (ln_pair_w[:, None] * Wv).astype(bf)
    wg_e = (ln_pair_w[:, None] * Wg).astype(bf)
    wb_e = (ln_bias_w[:, None] * Wb).astype(bf)
    wo_e = Wo.astype(bf)
    cq_e = (ln_pair_b @ (Wq * scaling)).astype(np.float32).reshape(D, 1)
    ck_e = (ln_pair_b @ (Wk * kscale)).astype(np.float32).reshape(D, 1)
    cv_e = (ln_pair_b @ Wv).astype(np.float32)
    cg_e = (bg + ln_pair_b @ Wg).astype(np.float32).reshape(D, 1)
    cb_e = (ln_bias_b @ Wb).astype(np.float32).reshape(H, 1)
    bo_f = np.asarray(bo, np.float32)
    has_bo = bool(np.any(bo_f != 0.0))
    has_cv = bool(np.any(cv_e != 0.0))
    bo_bcast = np.broadcast_to(bo_f, (D, D)).copy() if has_bo \
        else np.zeros((D, D), np.float32)
    # cvb row r = cv (v output features along the free axis, bcast over j)
    cvb_e = np.broadcast_to(cv_e[None, :], (D, D)).copy() if has_cv \
        else np.zeros((D, D), np.float32)

    common = dict(wq=wq_e, wk=wk_e, wv=wv_e, wg=wg_e, wo=wo_e, wb=wb_e,
                  cq=cq_e, ck=ck_e, cvb=cvb_e, cg=cg_e, cb=cb_e, bo_b=bo_bcast)
    in_maps = []
    for c in range(N_CORES):
        r0 = c * NL
        m = dict(common)
        m["xr"] = np.ascontiguousarray(pair[0, :, r0:r0 + NL, :], np.float32)
        m["xb"] = np.ascontiguousarray(bias[0, :, r0:r0 + NL, :], np.float32)
        in_maps.append(m)
    return in_maps, has_bo, has_cv


TRACE = False
LAST_EXEC_NS = None
LAST_TRACE_DIR = None
LAST_RES = None


def kernel(**inputs):
    global LAST_EXEC_NS, LAST_TRACE_DIR, LAST_RES
    inputs = {k: np.asarray(v) for k, v in inputs.items()}
    in_maps, has_bo, has_cv = _prep_inputs(**inputs)
    nc = build_program(has_bo, has_cv)
    res = run_bass_kernel_spmd(nc, in_maps, list(range(N_CORES)), trace=TRACE)
    if TRACE:
        LAST_EXEC_NS = res.exec_time_ns
        LAST_RES = res
    full = np.concatenate([res.results[c]["out"] for c in range(N_CORES)],
                          axis=0)[None]
    return full.astype(np.float32)


if __name__ == "__main__":
    nc = build_program(False, False)
    print("build ok")
